# revision 46
# baseline (speedup 1.0000x reference)
"""Lennard-Jones pair energies + per-atom segment sum on 8 Trainium2 cores.

Strategy (edge-partitioned per the sharding hint, ELL-style dense layout):

Host (sharding step): atoms are sorted by padded pair count and grouped into
chunks of 1024 (8 cores x 128 partitions); chunk i keeps L_i = max padded
count in the chunk plus 2-4 fixup slots so L_i % 4 == 0.  No pad atoms and
minimal slot padding.  Each core receives a partition-major fp16 buffer
[128, F_total]; every DMA is contiguous per partition.  Pad slots use
dist=RC (shifted LJ energy exactly 0); the fixup slots carry host-computed
distances whose pair energies sum to the column's additive constant
-L*e0/2, so the device-side reduce alone yields the final per-atom energy.
Within each device tile the columns are packed in four quarter-regions
(chunk slots block-split 4 ways) so the Tensor engine can fold the tile 4:1
by accumulating four identity matmuls into one PSUM bank.

Device: one activation-table preload (ln/exp share a table set), then per
tile: contiguous DMA, ACT ln (fp16->f32), ACT exp -> v = sqrt2*d^-6 (fp16),
DVE tensor_scalar u = v - 2b (4x rate), tensor_tensor bp = u*v (2x rate,
partly on GpSimd), PE 4:1 fold into PSUM (f32), DVE grouped tensor_reduce
per equal-L chunk run from PSUM, per-tile output DMA of [128, m] f32.

Host (unshard step): scatters per-atom results back to atom order.
"""

import math

import numpy as np

RC = 3.0
N_CORES = 8
P = 128
CH = N_CORES * P  # atoms per chunk
PAD_MULT = 1  # per-atom slot-count quantum

_E0 = 4.0 * ((1.0 / RC) ** 12 - (1.0 / RC) ** 6)
_B = math.sqrt(0.5)

# cumulative tile boundaries as fractions of total width (small first tile
# for pipeline ramp, graded-down end for a short tail)
TILE_FRACS = [0.05, 0.24, 0.53, 0.82, 0.95, 1.0]
GP_TT = 0.32  # fraction of the bp multiply given to GpSimd (early tiles)
GP_TILES = ()  # GpSimd measured ~5.6x slower per element than DVE-2x and
# gates the PE fold chains; with the PE fold, DVE has slack without it
DIRECT_TILES = (5,)  # last tile: reduce straight from bp (fewer sem hops)


def _merge_runs(Lc: np.ndarray, max_runs: int = 7, max_cost: int = 60000):
    """Round some chunks' L up to the next-larger run's L to cut the number
    of distinct L values. Lc is non-increasing (sorted desc)."""
    Lc = Lc.copy()
    while True:
        uniq = sorted(set(int(x) for x in Lc), reverse=True)
        if len(uniq) <= max_runs:
            break
        best = None
        for i in range(1, len(uniq)):
            src = uniq[i]
            dst = uniq[i - 1]
            m = int(np.sum(Lc == src))
            cost = m * CH * (dst - src)
            if best is None or cost < best[0]:
                best = (cost, src, dst)
        if best[0] > max_cost:
            break
        Lc[Lc == best[1]] = best[2]
    return Lc


def _chunk_geometry(idx: np.ndarray, n_atoms: int):
    counts = np.bincount(idx, minlength=n_atoms).astype(np.int64)
    perm = np.argsort(idx, kind="stable")
    starts = np.zeros(n_atoms + 1, np.int64)
    starts[1:] = np.cumsum(counts)
    q = ((counts + PAD_MULT - 1) // PAD_MULT) * PAD_MULT
    order = np.argsort(-q, kind="stable")
    n_chunks = (n_atoms + CH - 1) // CH
    n_pad = n_chunks * CH
    order_pad = np.full(n_pad, -1, np.int64)
    order_pad[:n_atoms] = order
    qs = np.where(order_pad >= 0, q[np.maximum(order_pad, 0)], 0)
    Lc = np.maximum(qs.reshape(n_chunks, CH).max(axis=1), PAD_MULT)
    Lc = _merge_runs(Lc)
    fix = 4 - (Lc % 4)  # 2 or 4 fixup slots -> Lp multiple of 4
    fix = np.where(fix == 0, 4, fix)
    Lp = Lc + fix
    return counts, perm, starts, order_pad, Lc, Lp, n_chunks


def _tile_plan(Lp):
    """Group chunks into device tiles at TILE_FRACS boundaries.

    Returns list of tiles (col_start, F, runs);
    runs = [(q_off, Lq, m, out_col)] over the tile's quarter-width layout,
    where Lq = Lp/4 and q_off is the column offset inside one quarter.
    """
    n = len(Lp)
    total = sum(Lp)
    bounds = []
    c0 = 0
    col = 0
    fi = 0
    for i in range(n):
        col += Lp[i]
        if fi < len(TILE_FRACS) - 1 and col >= TILE_FRACS[fi] * total:
            bounds.append((c0, i + 1))
            c0 = i + 1
            fi += 1
    if c0 < n:
        bounds.append((c0, n))
    tiles = []
    col = 0
    for c0, c1 in bounds:
        runs = []
        off = 0  # offset in quarter-width units
        j = c0
        while j < c1:
            k = j
            while k < c1 and Lp[k] == Lp[j]:
                k += 1
            runs.append((off, Lp[j] // 4, k - j, j))
            off += (Lp[j] // 4) * (k - j)
            j = k
        tiles.append((col, 4 * off, runs))
        col += 4 * off
    return tiles


def _build_layout(idx: np.ndarray, n_atoms: int, dist: np.ndarray):
    """Pack pairs into per-core partition-major fp16 tiles (quarter-split).

    Returns (packed, atom_of, Lp, n_chunks, tiles).
    """
    counts, perm, starts, order_pad, Lc, Lp, n_chunks = _chunk_geometry(
        idx, n_atoms
    )
    tiles = _tile_plan([int(x) for x in Lp])
    F_total = sum(F for _, F, _ in tiles)

    dist_sorted = dist[perm].astype(np.float16)
    packed = np.full((N_CORES, P, F_total), np.float16(RC), np.float16)
    Lmax = int(Lc.max())
    offs_max = np.arange(Lmax)
    for tcol, Ft, runs in tiles:
        Fq = Ft // 4
        for q_off, Lq, m, j0 in runs:
            for j in range(j0, j0 + m):
                a = order_pad[j * CH : (j + 1) * CH]
                L = int(Lc[j])
                nfix = int(Lp[j] - L)
                # each fixup slot contributes bp = -L*e0/(2*nfix)
                vf = _B + math.sqrt(0.5 - L * _E0 / (2.0 * nfix))
                df = (math.sqrt(2.0) / vf) ** (1.0 / 6.0)
                cnt = np.where(a >= 0, counts[np.maximum(a, 0)], 0)
                offs = offs_max[:L][None, :]
                valid = offs < cnt[:, None]
                src = starts[np.maximum(a, 0)][:, None] + offs
                block = np.full((CH, L + nfix), np.float16(RC), np.float16)
                block[:, :L][valid] = dist_sorted[src[valid]]
                block[:, L:] = np.float16(df)
                blk = block.reshape(N_CORES, P, L + nfix)
                o = tcol + q_off + (j - j0) * Lq
                for k in range(4):
                    packed[:, :, k * Fq + o : k * Fq + o + Lq] = blk[
                        :, :, k * Lq : (k + 1) * Lq
                    ]
    atom_of = order_pad.reshape(n_chunks, N_CORES, P)
    return packed, atom_of, [int(x) for x in Lp], n_chunks, tiles


def _build_bass_program(tiles, F_total, n_chunks):
    import concourse.bass as bass
    import concourse.tile as tile
    from concourse import bacc, mybir

    f32 = mybir.dt.float32
    f16 = mybir.dt.float16
    AF = mybir.ActivationFunctionType
    OP = mybir.AluOpType

    nc = bacc.Bacc(
        "TRN2",
        target_bir_lowering=False,
        debug=False,
        enable_asserts=False,
        num_devices=N_CORES,
    )
    din = nc.dram_tensor("dist_packed", [P, F_total], f16, kind="ExternalInput")
    dident = nc.dram_tensor("ident", [P, P], f16, kind="ExternalInput")
    dout = nc.dram_tensor("en_out", [P, n_chunks], f32, kind="ExternalOutput")

    # activation table set holding ln+exp together (one load for the whole
    # program instead of a 1.3us reload per function switch)
    set_id = 6
    try:
        from concourse.hw_specs import get_activation_tables

        for i, (_, funcs) in enumerate(get_activation_tables("TRN2").items()):
            if AF.Ln in funcs and AF.Exp in funcs:
                set_id = i
                break
    except Exception:
        pass

    ln_sqrt2 = 0.5 * math.log(2.0)

    with tile.TileContext(nc) as tc:
        with (
            tc.tile_pool(name="io", bufs=3) as io_pool,
            tc.tile_pool(name="t", bufs=3) as tpool,
            tc.tile_pool(name="v", bufs=6) as vpool,
            tc.tile_pool(name="u", bufs=4) as upool,
            tc.tile_pool(name="ps", bufs=4, space="PSUM") as pspool,
            tc.tile_pool(name="acc", bufs=1) as acc_pool,
        ):
            atl = mybir.InstLoadActFuncSet(
                name=nc.get_next_instruction_name(),
                ins=[],
                outs=[],
                act_func_set_id=set_id,
            )
            nc.scalar.add_instruction(atl)
            out_raw = acc_pool.tile([P, n_chunks], f32, tag="out_raw")
            lbias = acc_pool.tile([P, 1], f32, tag="lbias")
            nc.vector.memset(lbias[:], ln_sqrt2)
            # identity for the PE fold; issued via the idle GpSimd queue's
            # software DGE so it never delays the input stream on Sync
            ident = acc_pool.tile([P, P], f16, tag="ident")
            nc.gpsimd.dma_start(ident[:], dident.ap())
            n_tiles = len(tiles)
            for ti, (col, F, runs) in enumerate(tiles):
                Fq = F // 4
                use_gp = ti in GP_TILES
                d = io_pool.tile([P, F], f16, tag="d")
                nc.sync.dma_start(d[:], din.ap()[:, col : col + F])
                # t = ln(d) at f32 (exp amplifies ln error 6x)
                t = tpool.tile([P, F], f32, tag="t")
                nc.scalar.activation(t[:], d[:], AF.Ln)
                # v = sqrt2*d^-6 in fp16 in its own deep pool so the Scalar
                # engine never waits on Vector/GpSimd consumers (WAR)
                v = vpool.tile([P, F], f16, tag="v")
                nc.scalar.activation(
                    v[:], t[:], AF.Exp, bias=lbias[:], scale=-6.0
                )
                # bp = (v - 2b)*v ; en/2 = bp - e0/2 (constant folded into
                # the per-chunk fixup slots).  ts runs 4x, tt 2x; a slice
                # of tt goes to the otherwise-idle GpSimd engine.
                u = upool.tile([P, F], f16, tag="u")
                nc.vector.tensor_scalar(u[:], v[:], 2.0 * _B, None, OP.subtract)
                sp = (int(F * (1.0 - GP_TT)) & ~3) if use_gp else F
                nc.vector.tensor_tensor(
                    v[:, :sp], u[:, :sp], v[:, :sp], OP.mult
                )
                if sp < F:
                    nc.gpsimd.tensor_tensor(
                        v[:, sp:], u[:, sp:], v[:, sp:], OP.mult
                    )
                c0 = runs[0][3]
                c1 = runs[-1][3] + runs[-1][2]
                if ti in DIRECT_TILES:
                    # short dependency chain for the pipeline tail: reduce
                    # the quarter-split layout directly with an XY reduce
                    vq = v[:].rearrange("p (k x) -> p k x", k=4)
                    for q_off, Lq, m, out_col in runs:
                        nc.vector.tensor_reduce(
                            out_raw[:, out_col : out_col + m],
                            vq[:, :, q_off : q_off + m * Lq].rearrange(
                                "p k (b l) -> p b k l", l=Lq
                            ),
                            axis=mybir.AxisListType.XY,
                            op=OP.add,
                        )
                else:
                    # 4:1 fold on the Tensor engine: accumulate the four
                    # quarter-regions into one PSUM bank via identity matmuls
                    ps = pspool.tile([P, Fq], f32, tag="ps")
                    for k in range(4):
                        nc.tensor.matmul(
                            ps[:],
                            ident[:],
                            v[:, k * Fq : (k + 1) * Fq],
                            start=(k == 0),
                            stop=(k == 3),
                        )
                    for q_off, Lq, m, out_col in runs:
                        nc.vector.tensor_reduce(
                            out_raw[:, out_col : out_col + m],
                            ps[:, q_off : q_off + m * Lq].rearrange(
                                "p (b l) -> p b l", l=Lq
                            ),
                            axis=mybir.AxisListType.X,
                            op=OP.add,
                        )
                nc.sync.dma_start(dout.ap()[:, c0:c1], out_raw[:, c0:c1])
    nc.compile()
    return nc


def _prepare(inputs):
    dist = np.ascontiguousarray(np.asarray(inputs["dist"], dtype=np.float32))
    ind_2 = np.asarray(inputs["ind_2"])
    n_atoms = int(np.asarray(inputs["ind_1"]).shape[0])
    idx = ind_2[:, 0].astype(np.int64)

    packed, atom_of, Lp, n_chunks, tiles = _build_layout(idx, n_atoms, dist)
    F_total = packed.shape[2]
    ident = np.eye(P, dtype=np.float16)
    in_maps = [
        {"dist_packed": np.ascontiguousarray(packed[c]), "ident": ident}
        for c in range(N_CORES)
    ]
    nc = _build_bass_program(tiles, F_total, n_chunks)
    return nc, in_maps, (atom_of, n_atoms)


def _finish(res, meta):
    atom_of, n_atoms = meta
    out_full = np.zeros(n_atoms, np.float32)
    for c in range(N_CORES):
        dev = res.results[c]["en_out"]  # [P, n_chunks]
        a = atom_of[:, c, :]  # [n_chunks, P]
        valid = a >= 0
        out_full[a[valid]] = dev.T[valid]
    return out_full


def kernel(**inputs) -> np.ndarray:
    nc, in_maps, meta = _prepare(inputs)

    from concourse import bass_utils

    res = bass_utils.run_bass_kernel_spmd(
        nc, in_maps, core_ids=list(range(N_CORES))
    )
    return _finish(res, meta)


# revision 48
# speedup vs baseline: 1.0443x; 1.0443x over previous
"""Lennard-Jones pair energies + per-atom segment sum on 8 Trainium2 cores.

Strategy (edge-partitioned per the sharding hint, ELL-style dense layout):

Host (sharding step): atoms are sorted by padded pair count and grouped into
chunks of 1024 (8 cores x 128 partitions); chunk i keeps L_i = max padded
count in the chunk plus 2-4 fixup slots so L_i % 4 == 0.  No pad atoms and
minimal slot padding.  Each core receives a partition-major fp16 buffer
[128, F_total]; every DMA is contiguous per partition.  Pad slots use
dist=RC (shifted LJ energy exactly 0); the fixup slots carry host-computed
distances whose pair energies sum to the column's additive constant
-L*e0/2, so the device-side reduce alone yields the final per-atom energy.
Within each device tile the columns are packed in four quarter-regions
(chunk slots block-split 4 ways) so the Tensor engine can fold the tile 4:1
by accumulating four identity matmuls into one PSUM bank.

Device: one activation-table preload (ln/exp share a table set), then per
tile: contiguous DMA, ACT ln (fp16->f32), ACT exp -> v = sqrt2*d^-6 (fp16),
DVE tensor_scalar u = v - 2b (4x rate), tensor_tensor bp = u*v (2x rate,
partly on GpSimd), PE 4:1 fold into PSUM (f32), DVE grouped tensor_reduce
per equal-L chunk run from PSUM, per-tile output DMA of [128, m] f32.

Host (unshard step): scatters per-atom results back to atom order.
"""

import math

import numpy as np

RC = 3.0
N_CORES = 8
P = 128
CH = N_CORES * P  # atoms per chunk
PAD_MULT = 1  # per-atom slot-count quantum

_E0 = 4.0 * ((1.0 / RC) ** 12 - (1.0 / RC) ** 6)
_B = math.sqrt(0.5)

# cumulative tile boundaries as fractions of total width (small first tile
# for pipeline ramp, graded-down end for a short tail)
TILE_FRACS = [0.05, 0.24, 0.53, 0.81, 0.94, 1.0]
GP_TT = 0.32  # fraction of the bp multiply given to GpSimd (early tiles)
GP_TILES = ()  # GpSimd measured ~5.6x slower per element than DVE-2x and
# gates the PE fold chains; with the PE fold, DVE has slack without it
DIRECT_TILES = ()  # tiles that reduce straight from bp (fewer sem hops)


def _merge_runs(Lc: np.ndarray, max_runs: int = 7, max_cost: int = 60000):
    """Round some chunks' L up to the next-larger run's L to cut the number
    of distinct L values. Lc is non-increasing (sorted desc)."""
    Lc = Lc.copy()
    while True:
        uniq = sorted(set(int(x) for x in Lc), reverse=True)
        if len(uniq) <= max_runs:
            break
        best = None
        for i in range(1, len(uniq)):
            src = uniq[i]
            dst = uniq[i - 1]
            m = int(np.sum(Lc == src))
            cost = m * CH * (dst - src)
            if best is None or cost < best[0]:
                best = (cost, src, dst)
        if best[0] > max_cost:
            break
        Lc[Lc == best[1]] = best[2]
    return Lc


def _chunk_geometry(idx: np.ndarray, n_atoms: int):
    counts = np.bincount(idx, minlength=n_atoms).astype(np.int64)
    perm = np.argsort(idx, kind="stable")
    starts = np.zeros(n_atoms + 1, np.int64)
    starts[1:] = np.cumsum(counts)
    q = ((counts + PAD_MULT - 1) // PAD_MULT) * PAD_MULT
    order = np.argsort(-q, kind="stable")
    n_chunks = (n_atoms + CH - 1) // CH
    n_pad = n_chunks * CH
    order_pad = np.full(n_pad, -1, np.int64)
    order_pad[:n_atoms] = order
    qs = np.where(order_pad >= 0, q[np.maximum(order_pad, 0)], 0)
    Lc = np.maximum(qs.reshape(n_chunks, CH).max(axis=1), PAD_MULT)
    Lc = _merge_runs(Lc)
    fix = 4 - (Lc % 4)  # 2 or 4 fixup slots -> Lp multiple of 4
    fix = np.where(fix == 0, 4, fix)
    Lp = Lc + fix
    return counts, perm, starts, order_pad, Lc, Lp, n_chunks


def _tile_plan(Lp):
    """Group chunks into device tiles at TILE_FRACS boundaries.

    Returns list of tiles (col_start, F, runs);
    runs = [(q_off, Lq, m, out_col)] over the tile's quarter-width layout,
    where Lq = Lp/4 and q_off is the column offset inside one quarter.
    """
    n = len(Lp)
    total = sum(Lp)
    bounds = []
    c0 = 0
    col = 0
    fi = 0
    for i in range(n):
        col += Lp[i]
        if fi < len(TILE_FRACS) - 1 and col >= TILE_FRACS[fi] * total:
            bounds.append((c0, i + 1))
            c0 = i + 1
            fi += 1
    if c0 < n:
        bounds.append((c0, n))
    tiles = []
    col = 0
    for c0, c1 in bounds:
        runs = []
        off = 0  # offset in quarter-width units
        j = c0
        while j < c1:
            k = j
            while k < c1 and Lp[k] == Lp[j]:
                k += 1
            runs.append((off, Lp[j] // 4, k - j, j))
            off += (Lp[j] // 4) * (k - j)
            j = k
        tiles.append((col, 4 * off, runs))
        col += 4 * off
    return tiles


def _build_layout(idx: np.ndarray, n_atoms: int, dist: np.ndarray):
    """Pack pairs into per-core partition-major fp16 tiles (quarter-split).

    Returns (packed, atom_of, Lp, n_chunks, tiles).
    """
    counts, perm, starts, order_pad, Lc, Lp, n_chunks = _chunk_geometry(
        idx, n_atoms
    )
    tiles = _tile_plan([int(x) for x in Lp])
    F_total = sum(F for _, F, _ in tiles)

    dist_sorted = dist[perm].astype(np.float16)
    packed = np.full((N_CORES, P, F_total), np.float16(RC), np.float16)
    Lmax = int(Lc.max())
    offs_max = np.arange(Lmax)
    for tcol, Ft, runs in tiles:
        Fq = Ft // 4
        for q_off, Lq, m, j0 in runs:
            for j in range(j0, j0 + m):
                a = order_pad[j * CH : (j + 1) * CH]
                L = int(Lc[j])
                nfix = int(Lp[j] - L)
                # each fixup slot contributes bp = -L*e0/(2*nfix)
                vf = _B + math.sqrt(0.5 - L * _E0 / (2.0 * nfix))
                df = (math.sqrt(2.0) / vf) ** (1.0 / 6.0)
                cnt = np.where(a >= 0, counts[np.maximum(a, 0)], 0)
                offs = offs_max[:L][None, :]
                valid = offs < cnt[:, None]
                src = starts[np.maximum(a, 0)][:, None] + offs
                block = np.full((CH, L + nfix), np.float16(RC), np.float16)
                block[:, :L][valid] = dist_sorted[src[valid]]
                block[:, L:] = np.float16(df)
                blk = block.reshape(N_CORES, P, L + nfix)
                o = tcol + q_off + (j - j0) * Lq
                for k in range(4):
                    packed[:, :, k * Fq + o : k * Fq + o + Lq] = blk[
                        :, :, k * Lq : (k + 1) * Lq
                    ]
    atom_of = order_pad.reshape(n_chunks, N_CORES, P)
    return packed, atom_of, [int(x) for x in Lp], n_chunks, tiles


def _build_bass_program(tiles, F_total, n_chunks):
    import concourse.bass as bass
    import concourse.tile as tile
    from concourse import bacc, mybir

    f32 = mybir.dt.float32
    f16 = mybir.dt.float16
    AF = mybir.ActivationFunctionType
    OP = mybir.AluOpType

    nc = bacc.Bacc(
        "TRN2",
        target_bir_lowering=False,
        debug=False,
        enable_asserts=False,
        num_devices=N_CORES,
    )
    din = nc.dram_tensor("dist_packed", [P, F_total], f16, kind="ExternalInput")
    dident = nc.dram_tensor("ident", [P, P], f16, kind="ExternalInput")
    dout = nc.dram_tensor("en_out", [P, n_chunks], f32, kind="ExternalOutput")

    # activation table set holding ln+exp together (one load for the whole
    # program instead of a 1.3us reload per function switch)
    set_id = 6
    try:
        from concourse.hw_specs import get_activation_tables

        for i, (_, funcs) in enumerate(get_activation_tables("TRN2").items()):
            if AF.Ln in funcs and AF.Exp in funcs:
                set_id = i
                break
    except Exception:
        pass

    ln_sqrt2 = 0.5 * math.log(2.0)

    with tile.TileContext(nc) as tc:
        with (
            tc.tile_pool(name="io", bufs=3) as io_pool,
            tc.tile_pool(name="t", bufs=3) as tpool,
            tc.tile_pool(name="v", bufs=6) as vpool,
            tc.tile_pool(name="u", bufs=4) as upool,
            tc.tile_pool(name="ps", bufs=4, space="PSUM") as pspool,
            tc.tile_pool(name="acc", bufs=1) as acc_pool,
        ):
            atl = mybir.InstLoadActFuncSet(
                name=nc.get_next_instruction_name(),
                ins=[],
                outs=[],
                act_func_set_id=set_id,
            )
            nc.scalar.add_instruction(atl)
            out_raw = acc_pool.tile([P, n_chunks], f32, tag="out_raw")
            lbias = acc_pool.tile([P, 1], f32, tag="lbias")
            nc.vector.memset(lbias[:], ln_sqrt2)
            # identity for the PE fold; issued via the idle GpSimd queue's
            # software DGE so it never delays the input stream on Sync
            ident = acc_pool.tile([P, P], f16, tag="ident")
            nc.gpsimd.dma_start(ident[:], dident.ap())
            n_tiles = len(tiles)
            for ti, (col, F, runs) in enumerate(tiles):
                Fq = F // 4
                use_gp = ti in GP_TILES
                d = io_pool.tile([P, F], f16, tag="d")
                nc.sync.dma_start(d[:], din.ap()[:, col : col + F])
                # t = ln(d) at f32 (exp amplifies ln error 6x)
                t = tpool.tile([P, F], f32, tag="t")
                nc.scalar.activation(t[:], d[:], AF.Ln)
                # v = sqrt2*d^-6 in fp16 in its own deep pool so the Scalar
                # engine never waits on Vector/GpSimd consumers (WAR)
                v = vpool.tile([P, F], f16, tag="v")
                nc.scalar.activation(
                    v[:], t[:], AF.Exp, bias=lbias[:], scale=-6.0
                )
                # bp = (v - 2b)*v ; en/2 = bp - e0/2 (constant folded into
                # the per-chunk fixup slots).  ts runs 4x, tt 2x; a slice
                # of tt goes to the otherwise-idle GpSimd engine.
                u = upool.tile([P, F], f16, tag="u")
                nc.vector.tensor_scalar(u[:], v[:], 2.0 * _B, None, OP.subtract)
                sp = (int(F * (1.0 - GP_TT)) & ~3) if use_gp else F
                nc.vector.tensor_tensor(
                    v[:, :sp], u[:, :sp], v[:, :sp], OP.mult
                )
                if sp < F:
                    nc.gpsimd.tensor_tensor(
                        v[:, sp:], u[:, sp:], v[:, sp:], OP.mult
                    )
                c0 = runs[0][3]
                c1 = runs[-1][3] + runs[-1][2]
                if ti in DIRECT_TILES:
                    # short dependency chain for the pipeline tail: reduce
                    # the quarter-split layout directly with an XY reduce
                    vq = v[:].rearrange("p (k x) -> p k x", k=4)
                    for q_off, Lq, m, out_col in runs:
                        nc.vector.tensor_reduce(
                            out_raw[:, out_col : out_col + m],
                            vq[:, :, q_off : q_off + m * Lq].rearrange(
                                "p k (b l) -> p b k l", l=Lq
                            ),
                            axis=mybir.AxisListType.XY,
                            op=OP.add,
                        )
                else:
                    # 4:1 fold on the Tensor engine: accumulate the four
                    # quarter-regions into one PSUM bank via identity matmuls
                    ps = pspool.tile([P, Fq], f32, tag="ps")
                    for k in range(4):
                        nc.tensor.matmul(
                            ps[:],
                            ident[:],
                            v[:, k * Fq : (k + 1) * Fq],
                            start=(k == 0),
                            stop=(k == 3),
                        )
                    for q_off, Lq, m, out_col in runs:
                        nc.vector.tensor_reduce(
                            out_raw[:, out_col : out_col + m],
                            ps[:, q_off : q_off + m * Lq].rearrange(
                                "p (b l) -> p b l", l=Lq
                            ),
                            axis=mybir.AxisListType.X,
                            op=OP.add,
                        )
                nc.sync.dma_start(dout.ap()[:, c0:c1], out_raw[:, c0:c1])
    nc.compile()
    return nc


def _prepare(inputs):
    dist = np.ascontiguousarray(np.asarray(inputs["dist"], dtype=np.float32))
    ind_2 = np.asarray(inputs["ind_2"])
    n_atoms = int(np.asarray(inputs["ind_1"]).shape[0])
    idx = ind_2[:, 0].astype(np.int64)

    packed, atom_of, Lp, n_chunks, tiles = _build_layout(idx, n_atoms, dist)
    F_total = packed.shape[2]
    ident = np.eye(P, dtype=np.float16)
    in_maps = [
        {"dist_packed": np.ascontiguousarray(packed[c]), "ident": ident}
        for c in range(N_CORES)
    ]
    nc = _build_bass_program(tiles, F_total, n_chunks)
    return nc, in_maps, (atom_of, n_atoms)


def _finish(res, meta):
    atom_of, n_atoms = meta
    out_full = np.zeros(n_atoms, np.float32)
    for c in range(N_CORES):
        dev = res.results[c]["en_out"]  # [P, n_chunks]
        a = atom_of[:, c, :]  # [n_chunks, P]
        valid = a >= 0
        out_full[a[valid]] = dev.T[valid]
    return out_full


def kernel(**inputs) -> np.ndarray:
    nc, in_maps, meta = _prepare(inputs)

    from concourse import bass_utils

    res = bass_utils.run_bass_kernel_spmd(
        nc, in_maps, core_ids=list(range(N_CORES))
    )
    return _finish(res, meta)
